# revision 1
# baseline (speedup 1.0000x reference)
"""Deformable conv block kernel for 8 Trainium2 NeuronCores (v2).

Shards batch B=8 across 8 cores (pure data parallel). Per core:
  h1 = lrelu(conv3x3(hr, w1)); h2 = lrelu(conv3x3(h1, w2)); est = conv3x3(h2, w3)
  off = conv3x3(est, wo)  -> per-channel (dy, dx)
  sampled = bilinear(hr, grid + off)   (exact, 5x5 hat window)
  out = conv3x3(sampled, wc)
The lr_features path in the reference is dead (est[B:] depends only on hr).

v2 changes vs v1 (1.01 ms):
- kx-outer matmul ordering over PSUM groups of 4 tiles: one LDWEIGHTS per
  weight slice per group (6/group instead of 24) and a dense PE runway so
  the tensor engine can ramp to its high p-state.
- f16 coordinate/weight math in the bilinear (2x/4x DVE perf modes);
  offsets stored f16 in DRAM (halves that round-trip's traffic).
- hat weights via ACT Abs + one 4x-mode DVE tensor_scalar; no f32 ops.
- bilinear tap work split between DVE and GPSIMD (Pool): Pool owns the
  dy=2 x-chain and three y-multiplies.
- lrelu evac = ACT f32->f16 copy + Pool max(0.1x, x) (DVE stays free);
  final conv evac is an ACT f32 copy + flat DMA.
- padded-buffer borders zeroed once up front (rows + columns); evacs write
  interiors only with strided DMAs, killing the per-tile border memsets.
"""
import numpy as np
from contextlib import ExitStack

import concourse.bass as bass
import concourse.tile as tile
from concourse import bacc, mybir
from concourse.bass_utils import run_bass_kernel_spmd

F32 = mybir.dt.float32
F16 = mybir.dt.float16
ALU = mybir.AluOpType
ACTF = mybir.ActivationFunctionType

B, C, H, W = 8, 64, 160, 160
HW = H * W
WP = 162          # conv-padded width  (image col + 1)
HP = 164          # conv-padded rows   (image row + 1; rows 0,161..163 zero)
W16 = 164         # bilinear-padded width (image col + 2)
H16 = 164         # bilinear-padded rows  (image row + 2)
R_C = 32          # rows per pipeline band
R_B = 8           # bilinear rows per partition-half per block
FD = R_B * W      # 1280
N_CORES = 8
N_BAND = H // R_C          # 5 bands per stage

_CACHE = {}


def _conv_band(nc, pools, src, dst, wA, wB, M, kind, band, sid):
    """One 32-row band of a 3x3 conv stage; kx-outer over PSUM groups."""
    p_in, p_ps, p_ev = pools
    r0 = band * R_C
    in_t = p_in.tile([128, (R_C + 2) * WP], F16, name=f"cin{sid}_{band}",
                     tag="cin")
    iv = src.rearrange("c (r w) -> c r w", w=WP)
    nc.gpsimd.dma_start(in_t[0:64, :], iv[:, r0:r0 + R_C + 2, :])
    nc.gpsimd.dma_start(in_t[64:128, :], iv[:, r0 + 1:r0 + R_C + 3, :])
    it = in_t.rearrange("p (r w) -> p r w", w=WP)

    if kind in ("padded", "lrelu"):
        dv = dst.rearrange("c (r w) -> c r w", w=WP)

    # groups of PSUM tiles; each tile covers nr rows (N = nr*W columns)
    groups = [[(0, 3), (3, 3), (6, 3), (9, 3)],
              [(12, 3), (15, 3), (18, 3), (21, 3)],
              [(24, 3), (27, 3), (30, 2)]]
    for grp in groups:
        tiles = []
        for tl, nr in grp:
            ps = p_ps.tile([M, 480], F32, name=f"ps{sid}_{band}_{tl}",
                           tag="ps")
            tiles.append((tl, nr, ps))
        # kx-outer: reuse each weight slice across the group's tiles
        for kx in range(3):
            for tl, nr, ps in tiles:
                nc.tensor.matmul(
                    ps[:, 0:nr * W], wA[:, kx * M:(kx + 1) * M],
                    it[:, tl:tl + nr, kx:kx + 160],
                    start=(kx == 0), stop=False)
        for kx in range(3):
            for tl, nr, ps in tiles:
                nc.tensor.matmul(
                    ps[:, 0:nr * W], wB[0:64, kx * M:(kx + 1) * M],
                    it[0:64, tl + 2:tl + 2 + nr, kx:kx + 160],
                    start=False, stop=(kx == 2))

        for tl, nr, ps in tiles:
            N = nr * W
            pr = r0 + tl + 1
            if kind in ("lrelu", "padded"):
                # bordered tile: strided compute-engine write, flat DMA out
                ev = p_ev.tile([64, 3 * WP], F16, name=f"ev{sid}_{band}_{tl}",
                               tag="evP")
                e3 = ev.rearrange("p (r w) -> p r w", w=WP)
                nc.gpsimd.memset(e3[:, 0:nr, 0:1], 0.0)
                nc.gpsimd.memset(e3[:, 0:nr, 161:162], 0.0)
                e3i = e3[:, 0:nr, 1:161]
                if kind == "lrelu":
                    c2 = p_ev.tile([64, 480], F16,
                                   name=f"el{sid}_{band}_{tl}", tag="evL")
                    nc.scalar.activation(c2[:, 0:N], ps[:, 0:N], ACTF.Copy)
                    # lrelu(x) = max(0.1*x, x)
                    nc.vector.scalar_tensor_tensor(e3i, c2[:, 0:N], 0.1,
                                                   c2[:, 0:N], ALU.mult,
                                                   ALU.max)
                else:
                    nc.scalar.activation(e3i, ps[:, 0:N], ACTF.Copy)
                nc.sync.dma_start(dv[:, pr:pr + nr, :], ev[:, 0:nr * WP])
            elif kind == "offsets":
                offy_d, offx_d = dst
                ev = p_ev.tile([128, 480], F16, name=f"ev{sid}_{band}_{tl}",
                               tag="evO")
                nc.scalar.activation(ev[:, 0:N], ps[:, 0:N], ACTF.Copy)
                nc.sync.dma_start(
                    offy_d[:, (r0 + tl) * W:(r0 + tl + nr) * W], ev[0:64, 0:N])
                nc.sync.dma_start(
                    offx_d[:, (r0 + tl) * W:(r0 + tl + nr) * W],
                    ev[64:128, 0:N])
            else:  # "flat" f32 output
                ev = p_ev.tile([128, 480], F32, name=f"ev{sid}_{band}_{tl}",
                               tag="evF")
                nc.scalar.activation(ev[:, 0:N], ps[:, 0:N], ACTF.Copy)
                nc.sync.dma_start(
                    dst[:, (r0 + tl) * W:(r0 + tl + nr) * W], ev[:, 0:N])


def _bilinear_block(nc, bpools, hr16, offy_d, offx_d, samp_d,
                    jb0, jb1, xb0, xb1, biases, blk):
    """One 16-row block of bilinear sampling (8 rows per partition half).

    x-chains for dy != 2 and the y-accumulation run on DVE; the dy=2
    x-chain and the y-multiplies for dy in {1,2,3} run on Pool.
    """
    p_off, p_s, p_w, p_hr, p_acc = bpools
    rA = 2 * R_B * blk
    oy = p_off.tile([128, FD], F16, name=f"oy{blk}", tag="oy")
    ox = p_off.tile([128, FD], F16, name=f"ox{blk}", tag="ox")
    for half, r in ((0, rA), (1, rA + R_B)):
        nc.gpsimd.dma_start(oy[64 * half:64 * half + 64, :],
                            offy_d[:, r * W:(r + R_B) * W])
        nc.gpsimd.dma_start(ox[64 * half:64 * half + 64, :],
                            offx_d[:, r * W:(r + R_B) * W])
    hr_t = p_hr.tile([128, (R_B + 4) * W16], F16, name=f"hr{blk}", tag="hr")
    hv = hr16.rearrange("c (r w) -> c r w", w=W16)
    for half, r in ((0, rA), (1, rA + R_B)):
        nc.gpsimd.dma_start(hr_t[64 * half:64 * half + 64, :],
                            hv[:, r:r + R_B + 4, :])
    ht = hr_t.rearrange("p (r w) -> p r w", w=W16)
    # clipped residual coords: s = clip(off, -(row), 159-row), f16
    sy = p_s.tile([128, FD], F16, name=f"sy{blk}", tag="sy")
    sx = p_s.tile([128, FD], F16, name=f"sx{blk}", tag="sx")
    nc.vector.scalar_tensor_tensor(sy[:, :], jb0[:, :], float(-rA),
                                   oy[:, :], ALU.add, ALU.max)
    nc.vector.scalar_tensor_tensor(sy[:, :], jb1[:, :], float(-rA),
                                   sy[:, :], ALU.add, ALU.min)
    nc.vector.scalar_tensor_tensor(sx[:, :], xb0[:, :], 0.0,
                                   ox[:, :], ALU.add, ALU.max)
    nc.vector.scalar_tensor_tensor(sx[:, :], xb1[:, :], 0.0,
                                   sx[:, :], ALU.add, ALU.min)
    # negated hat weights: w'_d = min(|s-d|-1, 0) = -relu(1-|s-d|)
    wy = []
    wx = []
    for i, d in enumerate((-2, -1, 0, 1, 2)):
        for (s_t, w_list, ax) in ((sy, wy, "y"), (sx, wx, "x")):
            a = p_s.tile([128, FD], F16, name=f"a{ax}{i}_{blk}", tag="abs")
            nc.scalar.activation(a[:, :], s_t[:, :], ACTF.Abs,
                                 bias=biases[d][:, :])
            wt = p_w.tile([128, FD], F16, name=f"w{ax}{i}_{blk}",
                          tag=f"w{ax}{i}")
            nc.vector.tensor_scalar(wt[:, :], a[:, :], 1.0, 0.0,
                                    ALU.subtract, ALU.min)
            w_list.append(wt)

    acc = p_acc.tile([128, R_B * WP], F16, name=f"acc{blk}", tag="acc")
    a3 = acc.rearrange("p (r w) -> p r w", w=WP)
    nc.gpsimd.memset(a3[:, :, 0:1], 0.0)
    nc.gpsimd.memset(a3[:, :, 161:162], 0.0)
    acc_i = a3[:, :, 1:161]
    hs = p_acc.tile([128, FD], F16, name=f"hs{blk}", tag="hs")
    tmp = p_acc.tile([128, FD], F16, name=f"tmp{blk}", tag="tmp")
    hsp = p_acc.tile([128, FD], F16, name=f"hsp{blk}", tag="hsp")
    tmpp = p_acc.tile([128, FD], F16, name=f"tmpp{blk}", tag="tmpp")
    yp = [p_acc.tile([128, FD], F16, name=f"yp{k}_{blk}", tag=f"yp{k}")
          for k in range(3)]

    def xchain(eng, dy, hs_t, tmp_t):
        for dx in range(5):
            view = ht[:, dy:dy + R_B, dx:dx + 160]
            dst_t = hs_t if dx == 0 else tmp_t
            eng.tensor_mul(dst_t[:, :], wx[dx][:, :], view)
            if dx > 0:
                eng.tensor_add(hs_t[:, :], hs_t[:, :], tmp_t[:, :])

    # x-chains all on DVE; Pool takes only independent single-op y-muls
    # (dy=1..3). Distinct hs tiles per dy so a Pool read never blocks the
    # next DVE chain (WAR), and no long Pool chain gates the accumulation.
    # dy=0: acc = wy0 * hs
    xchain(nc.vector, 0, hs, tmp)
    nc.vector.tensor_mul(acc_i, wy[0][:, :], hs[:, :])
    # dy=1..3: DVE chain -> own tile, Pool y-mul -> yp[k]
    for k, (dy, hs_t) in enumerate(((1, hsp), (2, tmpp), (3, hs))):
        xchain(nc.vector, dy, hs_t, tmp)
        nc.gpsimd.tensor_mul(yp[k][:, :], wy[dy][:, :], hs_t[:, :])
    # dy=4 (DVE chain + DVE y-mul)
    xchain(nc.vector, 4, hsp, tmp)
    nc.vector.tensor_mul(tmp[:, :], wy[4][:, :], hsp[:, :])
    nc.vector.tensor_add(acc_i, acc_i, yp[0][:, :])
    nc.vector.tensor_add(acc_i, acc_i, yp[1][:, :])
    nc.vector.tensor_add(acc_i, acc_i, yp[2][:, :])
    nc.vector.tensor_add(acc_i, acc_i, tmp[:, :])

    nc.sync.dma_start(samp_d[:, (rA + 1) * WP:(rA + 1 + R_B) * WP],
                      acc[0:64, :])
    nc.sync.dma_start(
        samp_d[:, (rA + R_B + 1) * WP:(rA + 2 * R_B + 1) * WP],
        acc[64:128, :])


def build_program(debug_outputs=False):
    ikind = "ExternalOutput" if debug_outputs else "Internal"
    nc = bacc.Bacc("TRN2", target_bir_lowering=False, debug=False,
                   num_devices=N_CORES)
    xpad = nc.dram_tensor("xpad", [C, HP * WP], F16, kind="ExternalInput").ap()
    hr16 = nc.dram_tensor("hr16", [C, H16 * W16], F16,
                          kind="ExternalInput").ap()
    w_in = {}
    for s, m in (("w1", 64), ("w2", 64), ("w3", 64), ("wo", 128)):
        w_in[s + "A"] = nc.dram_tensor(s + "A", [128, 3 * m], F16,
                                       kind="ExternalInput").ap()
        w_in[s + "B"] = nc.dram_tensor(s + "B", [64, 3 * m], F16,
                                       kind="ExternalInput").ap()
    w_in["wcA"] = nc.dram_tensor("wcA", [128, 3 * 128], F16,
                                 kind="ExternalInput").ap()
    w_in["wcB"] = nc.dram_tensor("wcB", [64, 3 * 128], F16,
                                 kind="ExternalInput").ap()
    jb0_d = nc.dram_tensor("jb0", [128, FD], F16, kind="ExternalInput").ap()
    jb1_d = nc.dram_tensor("jb1", [128, FD], F16, kind="ExternalInput").ap()
    xb0_d = nc.dram_tensor("xb0", [128, FD], F16, kind="ExternalInput").ap()
    xb1_d = nc.dram_tensor("xb1", [128, FD], F16, kind="ExternalInput").ap()

    out = nc.dram_tensor("out", [128, HW], F32, kind="ExternalOutput").ap()

    h1p = nc.dram_tensor("h1p", [C, HP * WP], F16, kind=ikind).ap()
    h2p = nc.dram_tensor("h2p", [C, HP * WP], F16, kind=ikind).ap()
    estp = nc.dram_tensor("estp", [C, HP * WP], F16, kind=ikind).ap()
    offy_d = nc.dram_tensor("offy", [C, HW], F16, kind=ikind).ap()
    offx_d = nc.dram_tensor("offx", [C, HW], F16, kind=ikind).ap()
    samp_d = nc.dram_tensor("samp", [C, HP * WP], F16, kind=ikind).ap()

    with ExitStack() as ctx:
        tc = ctx.enter_context(tile.TileContext(nc))
        p_const = ctx.enter_context(tc.tile_pool(name="const", bufs=1))

        zrow = p_const.tile([64, 3 * WP], F32, name="zrow")
        nc.vector.memset(zrow[:, :], 0.0)
        zrow16 = zrow.bitcast(F16)
        for buf in (h1p, h2p, estp, samp_d):
            bv = buf.rearrange("c (r w) -> c r w", w=WP)
            nc.sync.dma_start(bv[:, 0:1, :], zrow16[:, 0:WP])
            nc.sync.dma_start(bv[:, 161:164, :], zrow16[:, 0:3 * WP])
            # zero the left/right pad columns of the interior rows
            nc.sync.dma_start(bv[:, 1:161, 0:1], zrow16[:, 0:160])
            nc.sync.dma_start(bv[:, 1:161, 161:162], zrow16[:, 0:160])

        wsb = {}
        for name, ap in w_in.items():
            t = p_const.tile(list(ap.shape), ap.dtype, name="w_" + name)
            nc.sync.dma_start(t[:, :], ap[:, :])
            wsb[name] = t
        jb0 = p_const.tile([128, FD], F16, name="jb0t")
        jb1 = p_const.tile([128, FD], F16, name="jb1t")
        xb0 = p_const.tile([128, FD], F16, name="xb0t")
        xb1 = p_const.tile([128, FD], F16, name="xb1t")
        for t, d in ((jb0, jb0_d), (jb1, jb1_d), (xb0, xb0_d), (xb1, xb1_d)):
            nc.sync.dma_start(t[:, :], d[:, :])

        p_in = ctx.enter_context(tc.tile_pool(name="c_in", bufs=4))
        p_ps = ctx.enter_context(tc.tile_pool(name="c_ps", bufs=8,
                                              space="PSUM"))
        p_ev = ctx.enter_context(tc.tile_pool(name="c_ev", bufs=5))
        pools = (p_in, p_ps, p_ev)

        p_off = ctx.enter_context(tc.tile_pool(name="b_off", bufs=2))
        p_s = ctx.enter_context(tc.tile_pool(name="b_s", bufs=2))
        p_w = ctx.enter_context(tc.tile_pool(name="b_w", bufs=2))
        p_hr = ctx.enter_context(tc.tile_pool(name="b_hr", bufs=2))
        p_acc = ctx.enter_context(tc.tile_pool(name="b_acc", bufs=2))
        bpools = (p_off, p_s, p_w, p_hr, p_acc)
        biases = {}
        for d in (-2, -1, 0, 1, 2):
            bt = p_w.tile([128, 1], F32, name=f"bias{d}", tag=f"bias{d}")
            nc.vector.memset(bt[:, :], float(-d))
            biases[d] = bt

        # interleaved band pipeline: at step i, stage s processes band i-s
        stages = [
            lambda b: _conv_band(nc, pools, xpad, h1p, wsb["w1A"],
                                 wsb["w1B"], 64, "lrelu", b, 1),
            lambda b: _conv_band(nc, pools, h1p, h2p, wsb["w2A"],
                                 wsb["w2B"], 64, "lrelu", b, 2),
            lambda b: _conv_band(nc, pools, h2p, estp, wsb["w3A"],
                                 wsb["w3B"], 64, "padded", b, 3),
            lambda b: _conv_band(nc, pools, estp, (offy_d, offx_d),
                                 wsb["woA"], wsb["woB"], 128, "offsets",
                                 b, 4),
            lambda b: [_bilinear_block(nc, bpools, hr16, offy_d, offx_d,
                                       samp_d, jb0, jb1, xb0, xb1, biases,
                                       2 * b + k) for k in (0, 1)],
            lambda b: _conv_band(nc, pools, samp_d, out, wsb["wcA"],
                                 wsb["wcB"], 128, "flat", b, 6),
        ]
        n_stage = len(stages)
        for i in range(N_BAND + n_stage - 1):
            for s in range(n_stage):
                b = i - s
                if 0 <= b < N_BAND:
                    stages[s](b)
    nc.compile()
    return nc


def _prep_weights(w, m, dtype):
    # w: (Cout, Cin, 3, 3) -> wA [128, 3*m] (ky=0/1 K-paired), wB [64, 3*m]
    wA = np.zeros((128, 3 * m), dtype=dtype)
    wB = np.zeros((64, 3 * m), dtype=dtype)
    for kx in range(3):
        wA[0:64, kx * m:(kx + 1) * m] = w[:, :, 0, kx].T
        wA[64:128, kx * m:(kx + 1) * m] = w[:, :, 1, kx].T
        wB[:, kx * m:(kx + 1) * m] = w[:, :, 2, kx].T
    return wA, wB


def _host_inputs(inputs):
    hr = np.asarray(inputs["hr_features"], dtype=np.float32)
    shared = {}
    for s, key, m in (("w1", "est_w1", 64), ("w2", "est_w2", 64),
                      ("w3", "est_w3", 64)):
        A, Bm = _prep_weights(np.asarray(inputs[key], np.float32), m,
                              np.float16)
        shared[s + "A"], shared[s + "B"] = A, Bm
    # offset conv: permute output channels to [dy c=0..63 | dx c=0..63]
    wo = np.asarray(inputs["offset_w"], np.float32)
    perm = np.concatenate([np.arange(0, 128, 2), np.arange(1, 128, 2)])
    A, Bm = _prep_weights(wo[perm], 128, np.float16)
    shared["woA"], shared["woB"] = A, Bm
    A, Bm = _prep_weights(np.asarray(inputs["conv1_w"], np.float32), 128,
                          np.float16)
    shared["wcA"], shared["wcB"] = A, Bm

    j = np.arange(R_B, dtype=np.float32)
    jcol = np.repeat(j, W)[None, :].repeat(128, 0)
    jcol[64:, :] += R_B
    shared["jb0"] = -jcol
    shared["jb1"] = 159.0 - jcol
    x = np.arange(W, dtype=np.float32)
    xcol = np.tile(x, R_B)[None, :].repeat(128, 0)
    shared["xb0"] = -xcol
    shared["xb1"] = 159.0 - xcol
    for k in ("jb0", "jb1", "xb0", "xb1"):
        shared[k] = np.ascontiguousarray(shared[k], dtype=np.float16)

    in_maps = []
    for b in range(B):
        m = dict(shared)
        xpad = np.zeros((C, HP, WP), np.float16)
        xpad[:, 1:161, 1:161] = hr[b]
        m["xpad"] = xpad.reshape(C, HP * WP)
        hr16 = np.zeros((C, H16, W16), np.float16)
        hr16[:, 2:162, 2:162] = hr[b].astype(np.float16)
        m["hr16"] = hr16.reshape(C, H16 * W16)
        in_maps.append(m)
    return in_maps


def kernel(**inputs):
    if "nc" not in _CACHE:
        _CACHE["nc"] = build_program()
    nc = _CACHE["nc"]
    in_maps = _host_inputs(inputs)
    res = run_bass_kernel_spmd(nc, in_maps, list(range(N_CORES)))
    out = np.stack([res.results[b]["out"].reshape(128, H, W)
                    for b in range(B)])
    return out.astype(np.float32)



# revision 5
# speedup vs baseline: 1.4588x; 1.4588x over previous
"""Deformable conv block kernel for 8 Trainium2 NeuronCores (v3).

Shards batch B=8 across 8 cores (pure data parallel). Per core:
  h1 = lrelu(conv3x3(hr, w1)); h2 = lrelu(conv3x3(h1, w2)); est = conv3x3(h2, w3)
  off = conv3x3(est, wo)  -> per-channel (dy, dx)
  sampled = bilinear(hr, grid + off)   (exact, 5x5 hat window)
  out = conv3x3(sampled, wc)
The lr_features path in the reference is dead (est[B:] depends only on hr).

v3 changes vs v2 (919 us):
- M=64 convs col-tiled: pairs of row-tiles stream concurrently through the
  two 64-col halves of the PE array (tile_position via psum base partition),
  halving their stream time.
- lrelu evac is a single ACT Lrelu op (psum f32 -> f16), freeing DVE.
- hr16 is replicate-padded (host side), which makes clipped-bilinear equal
  to unclipped hat-window sampling: the 4 DVE clip ops and the jb/xb
  coordinate tables are gone.
- bilinear blocks are 32 rows (R_B=16 per partition half, FD=2560): fewer,
  wider DVE ops; one block per conv band.
- redundant border-column zero DMAs removed (evacs write full padded rows);
  ~82k 2-byte DMA descriptors gone.
- final output stored f16 (host casts back to f32).
- conv input loads issue from the ACT queue (HWDGE) instead of Pool.
- pool does 4 of 5 bilinear y-muls; last y-mul on DVE so the accumulation
  tail never waits on Pool.
"""
import numpy as np
from contextlib import ExitStack

import concourse.bass as bass
import concourse.tile as tile
from concourse import bacc, mybir
from concourse.bass_utils import run_bass_kernel_spmd

F32 = mybir.dt.float32
F16 = mybir.dt.float16
ALU = mybir.AluOpType
ACTF = mybir.ActivationFunctionType

B, C, H, W = 8, 64, 160, 160
HW = H * W
WP = 162          # conv-padded width  (image col + 1)
HP = 164          # conv-padded rows   (image row + 1; rows 0,161..163 zero)
W16 = 164         # bilinear-padded width (image col + 2, replicate)
H16 = 164         # bilinear-padded rows
R_C = 32          # rows per pipeline band
R_B = 16          # bilinear rows per partition-half per block
FD = R_B * W      # 2560
N_CORES = 8
N_BAND = H // R_C          # 5 bands per stage

_CACHE = {}


def _conv_band_m64(nc, pools, src, dst, wA, wB, kind, band, sid):
    """One 32-row band of a 3x3 conv with Cout=64; pairs of row-tiles are
    col-tiled onto the two 64-wide halves of the PE array."""
    p_in, p_ps, p_ev = pools
    M = 64
    r0 = band * R_C
    in_t = p_in.tile([128, (R_C + 2) * WP], F16, name=f"cin{sid}_{band}",
                     tag="cin")
    iv = src.rearrange("c (r w) -> c r w", w=WP)
    nc.scalar.dma_start(in_t[0:64, :], iv[:, r0:r0 + R_C + 2, :])
    nc.scalar.dma_start(in_t[64:128, :], iv[:, r0 + 1:r0 + R_C + 3, :])
    it = in_t.rearrange("p (r w) -> p r w", w=WP)
    dv = dst.rearrange("c (r w) -> c r w", w=WP)

    # groups of tile-pairs; each pair shares one [128,480] psum tile
    # (lo half = tile A, hi half = tile B -> concurrent col groups)
    groups = [[(0, 3, 3, 3), (6, 3, 9, 3)],
              [(12, 3, 15, 3), (18, 3, 21, 3)],
              [(24, 3, 27, 3), (30, 2, None, 0)]]
    for gi, grp in enumerate(groups):
        pairs = []
        for tlA, nrA, tlB, nrB in grp:
            ps = p_ps.tile([128, 480], F32, name=f"ps{sid}_{band}_{tlA}",
                           tag="ps")
            pairs.append((tlA, nrA, tlB, nrB, ps))
        # kx-outer over the group's pairs; A->cols 0-63, B->cols 64-127
        for kx in range(3):
            for tlA, nrA, tlB, nrB, ps in pairs:
                nc.tensor.matmul(
                    ps[0:64, 0:nrA * W], wA[:, kx * M:(kx + 1) * M],
                    it[:, tlA:tlA + nrA, kx:kx + 160],
                    start=(kx == 0), stop=False)
                if tlB is not None:
                    nc.tensor.matmul(
                        ps[64:128, 0:nrB * W], wA[:, kx * M:(kx + 1) * M],
                        it[:, tlB:tlB + nrB, kx:kx + 160],
                        start=(kx == 0), stop=False)
        for kx in range(3):
            for tlA, nrA, tlB, nrB, ps in pairs:
                nc.tensor.matmul(
                    ps[0:64, 0:nrA * W], wB[0:64, kx * M:(kx + 1) * M],
                    it[0:64, tlA + 2:tlA + 2 + nrA, kx:kx + 160],
                    start=False, stop=(kx == 2))
                if tlB is not None:
                    nc.tensor.matmul(
                        ps[64:128, 0:nrB * W], wB[0:64, kx * M:(kx + 1) * M],
                        it[0:64, tlB + 2:tlB + 2 + nrB, kx:kx + 160],
                        start=False, stop=(kx == 2))

        for tlA, nrA, tlB, nrB, ps in pairs:
            ev = p_ev.tile([128, 3 * WP], F16, name=f"ev{sid}_{band}_{tlA}",
                           tag="evP")
            e3 = ev.rearrange("p (r w) -> p r w", w=WP)
            nrm = max(nrA, nrB)
            nc.gpsimd.memset(e3[:, 0:nrm, 0:1], 0.0)
            nc.gpsimd.memset(e3[:, 0:nrm, 161:162], 0.0)
            halves = [(0, tlA, nrA)]
            if tlB is not None:
                halves.append((64, tlB, nrB))
            for p0, tl, nr in halves:
                N = nr * W
                dst_i = e3[p0:p0 + 64, 0:nr, 1:161]
                if kind == "lrelu":
                    # Prelu honors alpha; Lrelu's slope is hardwired 0.01
                    nc.scalar.activation(dst_i, ps[p0:p0 + 64, 0:N],
                                         ACTF.Prelu, alpha=0.1)
                else:  # "padded"
                    nc.scalar.activation(dst_i, ps[p0:p0 + 64, 0:N],
                                         ACTF.Copy)
                pr = r0 + tl + 1
                nc.sync.dma_start(dv[:, pr:pr + nr, :],
                                  ev[p0:p0 + 64, 0:nr * WP])


def _conv_band_m128(nc, pools, src, dst, wA, wB, kind, band, sid):
    """One 32-row band of a 3x3 conv stage with Cout=128 (M=128)."""
    p_in, p_ps, p_ev = pools
    M = 128
    r0 = band * R_C
    in_t = p_in.tile([128, (R_C + 2) * WP], F16, name=f"cin{sid}_{band}",
                     tag="cin")
    iv = src.rearrange("c (r w) -> c r w", w=WP)
    nc.scalar.dma_start(in_t[0:64, :], iv[:, r0:r0 + R_C + 2, :])
    nc.scalar.dma_start(in_t[64:128, :], iv[:, r0 + 1:r0 + R_C + 3, :])
    it = in_t.rearrange("p (r w) -> p r w", w=WP)

    groups = [[(0, 3), (3, 3), (6, 3), (9, 3)],
              [(12, 3), (15, 3), (18, 3), (21, 3)],
              [(24, 3), (27, 3), (30, 2)]]
    for grp in groups:
        tiles = []
        for tl, nr in grp:
            ps = p_ps.tile([M, 480], F32, name=f"ps{sid}_{band}_{tl}",
                           tag="ps")
            tiles.append((tl, nr, ps))
        for kx in range(3):
            for tl, nr, ps in tiles:
                nc.tensor.matmul(
                    ps[:, 0:nr * W], wA[:, kx * M:(kx + 1) * M],
                    it[:, tl:tl + nr, kx:kx + 160],
                    start=(kx == 0), stop=False)
        for kx in range(3):
            for tl, nr, ps in tiles:
                nc.tensor.matmul(
                    ps[:, 0:nr * W], wB[0:64, kx * M:(kx + 1) * M],
                    it[0:64, tl + 2:tl + 2 + nr, kx:kx + 160],
                    start=False, stop=(kx == 2))

        for tl, nr, ps in tiles:
            N = nr * W
            if kind == "offsets":
                offy_d, offx_d = dst
                ev = p_ev.tile([128, 480], F16, name=f"ev{sid}_{band}_{tl}",
                               tag="evO")
                nc.scalar.activation(ev[:, 0:N], ps[:, 0:N], ACTF.Copy)
                nc.sync.dma_start(
                    offy_d[:, (r0 + tl) * W:(r0 + tl + nr) * W], ev[0:64, 0:N])
                nc.sync.dma_start(
                    offx_d[:, (r0 + tl) * W:(r0 + tl + nr) * W],
                    ev[64:128, 0:N])
            else:  # "flat" f16 output
                ev = p_ev.tile([128, 480], F16, name=f"ev{sid}_{band}_{tl}",
                               tag="evF")
                nc.scalar.activation(ev[:, 0:N], ps[:, 0:N], ACTF.Copy)
                nc.sync.dma_start(
                    dst[:, (r0 + tl) * W:(r0 + tl + nr) * W], ev[:, 0:N])


def _bilinear_block(nc, bpools, hr16, offy_d, offx_d, samp_d, biases, blk):
    """One 32-row block of bilinear sampling (16 rows per partition half).

    Residual coords = raw offsets (no clipping): hr16 is replicate-padded,
    which makes the unclipped 5x5 hat window exactly equal to the
    reference's clipped bilinear for |off| <= 2.
    """
    p_off, p_s, p_w, p_hr, p_acc = bpools
    rA = 2 * R_B * blk
    oy = p_off.tile([128, FD], F16, name=f"oy{blk}", tag="oy")
    ox = p_off.tile([128, FD], F16, name=f"ox{blk}", tag="ox")
    for half, r in ((0, rA), (1, rA + R_B)):
        nc.gpsimd.dma_start(oy[64 * half:64 * half + 64, :],
                            offy_d[:, r * W:(r + R_B) * W])
        nc.gpsimd.dma_start(ox[64 * half:64 * half + 64, :],
                            offx_d[:, r * W:(r + R_B) * W])
    hr_t = p_hr.tile([128, (R_B + 4) * W16], F16, name=f"hr{blk}", tag="hr")
    hv = hr16.rearrange("c (r w) -> c r w", w=W16)
    for half, r in ((0, rA), (1, rA + R_B)):
        nc.gpsimd.dma_start(hr_t[64 * half:64 * half + 64, :],
                            hv[:, r:r + R_B + 4, :])
    ht = hr_t.rearrange("p (r w) -> p r w", w=W16)

    # negated hat weights: w'_d = min(|s-d|-1, 0) = -relu(1-|s-d|)
    # wx persist across all chains (5 tags); wy[dy] is built just before
    # chain dy on a 2-ring (its Pool y-mul consumes it right after, so a
    # ring entry is free again two chains later).
    DVALS = (-2, -1, 0, 1, 2)

    def build_w(src, i, ax, tag):
        a = p_s.tile([128, FD], F16, name=f"a{ax}{i}_{blk}", tag="abs")
        nc.scalar.activation(a[:, :], src[:, :], ACTF.Abs,
                             bias=biases[DVALS[i]][:, :])
        wt = p_w.tile([128, FD], F16, name=f"w{ax}{i}_{blk}", tag=tag)
        nc.vector.tensor_scalar(wt[:, :], a[:, :], 1.0, 0.0,
                                ALU.subtract, ALU.min)
        return wt

    wx = [build_w(ox, i, "x", f"wx{i}") for i in range(5)]

    hs = [p_acc.tile([128, FD], F16, name=f"hs{k}_{blk}", tag=f"hs{k}")
          for k in range(3)]
    tmp = p_acc.tile([128, FD], F16, name=f"tmp{blk}", tag="tmp")
    tmp2 = p_acc.tile([128, FD], F16, name=f"tmp2{blk}", tag="tmp2")
    yp = [p_acc.tile([128, FD], F16, name=f"yp{k}_{blk}", tag=f"yp{k}")
          for k in range(4)]

    def xchain(dy, hs_t):
        for dx in range(5):
            view = ht[:, dy:dy + R_B, dx:dx + 160]
            dst_t = hs_t if dx == 0 else tmp
            nc.vector.tensor_mul(dst_t[:, :], wx[dx][:, :], view)
            if dx > 0:
                nc.vector.tensor_add(hs_t[:, :], hs_t[:, :], tmp[:, :])

    # dy=0..3: DVE chain -> hs ring; Pool y-mul -> yp[dy]
    for dy in range(4):
        wyt = build_w(oy, dy, "y", f"wy{dy % 2}")
        xchain(dy, hs[dy % 3])
        nc.gpsimd.tensor_mul(yp[dy][:, :], wyt[:, :], hs[dy % 3][:, :])
    # dy=4: DVE chain + DVE y-mul (so the tail never waits on Pool)
    wyt = build_w(oy, 4, "y", "wy0")
    xchain(4, hs[1])
    nc.vector.tensor_mul(tmp2[:, :], wyt[:, :], hs[1][:, :])

    acc = p_acc.tile([128, R_B * WP], F16, name=f"acc{blk}", tag="acc")
    a3 = acc.rearrange("p (r w) -> p r w", w=WP)
    nc.gpsimd.memset(a3[:, :, 0:1], 0.0)
    nc.gpsimd.memset(a3[:, :, 161:162], 0.0)
    acc_i = a3[:, :, 1:161]
    nc.vector.tensor_add(acc_i, yp[0][:, :], yp[1][:, :])
    nc.vector.tensor_add(acc_i, acc_i, yp[2][:, :])
    nc.vector.tensor_add(acc_i, acc_i, yp[3][:, :])
    nc.vector.tensor_add(acc_i, acc_i, tmp2[:, :])

    nc.sync.dma_start(samp_d[:, (rA + 1) * WP:(rA + 1 + R_B) * WP],
                      acc[0:64, :])
    nc.sync.dma_start(
        samp_d[:, (rA + R_B + 1) * WP:(rA + 2 * R_B + 1) * WP],
        acc[64:128, :])


def build_program(debug_outputs=False):
    ikind = "ExternalOutput" if debug_outputs else "Internal"
    nc = bacc.Bacc("TRN2", target_bir_lowering=False, debug=False,
                   num_devices=N_CORES)
    xpad = nc.dram_tensor("xpad", [C, HP * WP], F16, kind="ExternalInput").ap()
    hr16 = nc.dram_tensor("hr16", [C, H16 * W16], F16,
                          kind="ExternalInput").ap()
    w_in = {}
    for s, m in (("w1", 64), ("w2", 64), ("w3", 64), ("wo", 128)):
        w_in[s + "A"] = nc.dram_tensor(s + "A", [128, 3 * m], F16,
                                       kind="ExternalInput").ap()
        w_in[s + "B"] = nc.dram_tensor(s + "B", [64, 3 * m], F16,
                                       kind="ExternalInput").ap()
    w_in["wcA"] = nc.dram_tensor("wcA", [128, 3 * 128], F16,
                                 kind="ExternalInput").ap()
    w_in["wcB"] = nc.dram_tensor("wcB", [64, 3 * 128], F16,
                                 kind="ExternalInput").ap()

    out = nc.dram_tensor("out", [128, HW], F16, kind="ExternalOutput").ap()

    h1p = nc.dram_tensor("h1p", [C, HP * WP], F16, kind=ikind).ap()
    h2p = nc.dram_tensor("h2p", [C, HP * WP], F16, kind=ikind).ap()
    estp = nc.dram_tensor("estp", [C, HP * WP], F16, kind=ikind).ap()
    offy_d = nc.dram_tensor("offy", [C, HW], F16, kind=ikind).ap()
    offx_d = nc.dram_tensor("offx", [C, HW], F16, kind=ikind).ap()
    samp_d = nc.dram_tensor("samp", [C, HP * WP], F16, kind=ikind).ap()

    with ExitStack() as ctx:
        tc = ctx.enter_context(tile.TileContext(nc))
        p_const = ctx.enter_context(tc.tile_pool(name="const", bufs=1))

        zrow = p_const.tile([64, 3 * WP], F32, name="zrow")
        nc.vector.memset(zrow[:, :], 0.0)
        zrow16 = zrow.bitcast(F16)
        for buf in (h1p, h2p, estp, samp_d):
            bv = buf.rearrange("c (r w) -> c r w", w=WP)
            nc.sync.dma_start(bv[:, 0:1, :], zrow16[:, 0:WP])
            nc.sync.dma_start(bv[:, 161:164, :], zrow16[:, 0:3 * WP])

        wsb = {}
        for name, ap in w_in.items():
            t = p_const.tile(list(ap.shape), ap.dtype, name="w_" + name)
            nc.sync.dma_start(t[:, :], ap[:, :])
            wsb[name] = t
        biases = {}
        for d in (-2, -1, 0, 1, 2):
            bt = p_const.tile([128, 1], F32, name=f"bias{d}")
            nc.vector.memset(bt[:, :], float(-d))
            biases[d] = bt

        p_in = ctx.enter_context(tc.tile_pool(name="c_in", bufs=4))
        p_ps = ctx.enter_context(tc.tile_pool(name="c_ps", bufs=8,
                                              space="PSUM"))
        p_ev = ctx.enter_context(tc.tile_pool(name="c_ev", bufs=6))
        pools = (p_in, p_ps, p_ev)

        p_off = ctx.enter_context(tc.tile_pool(name="b_off", bufs=2))
        p_s = ctx.enter_context(tc.tile_pool(name="b_s", bufs=2))
        p_w = ctx.enter_context(tc.tile_pool(name="b_w", bufs=1))
        p_hr = ctx.enter_context(tc.tile_pool(name="b_hr", bufs=2))
        p_acc = ctx.enter_context(tc.tile_pool(name="b_acc", bufs=1))
        bpools = (p_off, p_s, p_w, p_hr, p_acc)

        # stage s processes band i - s at iteration i; issue order puts the
        # final conv first (its deps resolved last iteration) and the big
        # DVE bilinear chain last.
        stages = {
            0: lambda b: _conv_band_m64(nc, pools, xpad, h1p, wsb["w1A"],
                                        wsb["w1B"], "lrelu", b, 1),
            1: lambda b: _conv_band_m64(nc, pools, h1p, h2p, wsb["w2A"],
                                        wsb["w2B"], "lrelu", b, 2),
            2: lambda b: _conv_band_m64(nc, pools, h2p, estp, wsb["w3A"],
                                        wsb["w3B"], "padded", b, 3),
            3: lambda b: _conv_band_m128(nc, pools, estp, (offy_d, offx_d),
                                         wsb["woA"], wsb["woB"], "offsets",
                                         b, 4),
            4: lambda b: _bilinear_block(nc, bpools, hr16, offy_d, offx_d,
                                         samp_d, biases, b),
            6: lambda b: _conv_band_m128(nc, pools, samp_d, out, wsb["wcA"],
                                         wsb["wcB"], "flat", b, 6),
        }
        issue_order = [6, 0, 1, 2, 3, 4]
        n_stage = 7
        for i in range(N_BAND + n_stage - 1):
            for s in issue_order:
                b = i - s
                if 0 <= b < N_BAND:
                    stages[s](b)
    nc.compile()
    return nc


def _prep_weights(w, m, dtype):
    # w: (Cout, Cin, 3, 3) -> wA [128, 3*m] (ky=0/1 K-paired), wB [64, 3*m]
    wA = np.zeros((128, 3 * m), dtype=dtype)
    wB = np.zeros((64, 3 * m), dtype=dtype)
    for kx in range(3):
        wA[0:64, kx * m:(kx + 1) * m] = w[:, :, 0, kx].T
        wA[64:128, kx * m:(kx + 1) * m] = w[:, :, 1, kx].T
        wB[:, kx * m:(kx + 1) * m] = w[:, :, 2, kx].T
    return wA, wB


def _host_inputs(inputs):
    hr = np.asarray(inputs["hr_features"], dtype=np.float32)
    shared = {}
    for s, key, m in (("w1", "est_w1", 64), ("w2", "est_w2", 64),
                      ("w3", "est_w3", 64)):
        A, Bm = _prep_weights(np.asarray(inputs[key], np.float32), m,
                              np.float16)
        shared[s + "A"], shared[s + "B"] = A, Bm
    # offset conv: permute output channels to [dy c=0..63 | dx c=0..63]
    wo = np.asarray(inputs["offset_w"], np.float32)
    perm = np.concatenate([np.arange(0, 128, 2), np.arange(1, 128, 2)])
    A, Bm = _prep_weights(wo[perm], 128, np.float16)
    shared["woA"], shared["woB"] = A, Bm
    A, Bm = _prep_weights(np.asarray(inputs["conv1_w"], np.float32), 128,
                          np.float16)
    shared["wcA"], shared["wcB"] = A, Bm

    in_maps = []
    for b in range(B):
        m = dict(shared)
        xpad = np.zeros((C, HP, WP), np.float16)
        xpad[:, 1:161, 1:161] = hr[b]
        m["xpad"] = xpad.reshape(C, HP * WP)
        # replicate padding: unclipped hat-window == clipped bilinear
        hr16 = np.pad(hr[b].astype(np.float16), ((0, 0), (2, 2), (2, 2)),
                      mode="edge")
        m["hr16"] = hr16.reshape(C, H16 * W16)
        in_maps.append(m)
    return in_maps


def kernel(**inputs):
    if "nc" not in _CACHE:
        _CACHE["nc"] = build_program()
    nc = _CACHE["nc"]
    in_maps = _host_inputs(inputs)
    res = run_bass_kernel_spmd(nc, in_maps, list(range(N_CORES)))
    out = np.stack([res.results[b]["out"].reshape(128, H, W)
                    for b in range(B)])
    return out.astype(np.float32)


# revision 9
# speedup vs baseline: 1.4669x; 1.0055x over previous
"""Deformable conv block kernel for 8 Trainium2 NeuronCores (v3).

Shards batch B=8 across 8 cores (pure data parallel). Per core:
  h1 = lrelu(conv3x3(hr, w1)); h2 = lrelu(conv3x3(h1, w2)); est = conv3x3(h2, w3)
  off = conv3x3(est, wo)  -> per-channel (dy, dx)
  sampled = bilinear(hr, grid + off)   (exact, 5x5 hat window)
  out = conv3x3(sampled, wc)
The lr_features path in the reference is dead (est[B:] depends only on hr).

v3 changes vs v2 (919 us):
- M=64 convs col-tiled: pairs of row-tiles stream concurrently through the
  two 64-col halves of the PE array (tile_position via psum base partition),
  halving their stream time.
- lrelu evac is a single ACT Lrelu op (psum f32 -> f16), freeing DVE.
- hr16 is replicate-padded (host side), which makes clipped-bilinear equal
  to unclipped hat-window sampling: the 4 DVE clip ops and the jb/xb
  coordinate tables are gone.
- bilinear blocks are 32 rows (R_B=16 per partition half, FD=2560): fewer,
  wider DVE ops; one block per conv band.
- redundant border-column zero DMAs removed (evacs write full padded rows);
  ~82k 2-byte DMA descriptors gone.
- final output stored f16 (host casts back to f32).
- conv input loads issue from the ACT queue (HWDGE) instead of Pool.
- pool does 4 of 5 bilinear y-muls; last y-mul on DVE so the accumulation
  tail never waits on Pool.
"""
import numpy as np
from contextlib import ExitStack

import concourse.bass as bass
import concourse.tile as tile
from concourse import bacc, mybir
from concourse.bass_utils import run_bass_kernel_spmd

F32 = mybir.dt.float32
F16 = mybir.dt.float16
ALU = mybir.AluOpType
ACTF = mybir.ActivationFunctionType

B, C, H, W = 8, 64, 160, 160
HW = H * W
WP = 162          # conv-padded width  (image col + 1)
HP = 164          # conv-padded rows   (image row + 1; rows 0,161..163 zero)
W16 = 164         # bilinear-padded width (image col + 2, replicate)
H16 = 164         # bilinear-padded rows
R_C = 32          # rows per pipeline band
R_B = 16          # bilinear rows per partition-half per block
FD = R_B * W      # 2560
N_CORES = 8
N_BAND = H // R_C          # 5 bands per stage

_CACHE = {}


def _conv_band_m64(nc, pools, src, dst, wA, wB, kind, band, sid):
    """One 32-row band of a 3x3 conv with Cout=64; pairs of row-tiles are
    col-tiled onto the two 64-wide halves of the PE array."""
    p_in, p_ps, p_ev = pools
    M = 64
    r0 = band * R_C
    in_t = p_in.tile([128, (R_C + 2) * WP], F16, name=f"cin{sid}_{band}",
                     tag="cin")
    iv = src.rearrange("c (r w) -> c r w", w=WP)
    nc.scalar.dma_start(in_t[0:64, :], iv[:, r0:r0 + R_C + 2, :])
    nc.scalar.dma_start(in_t[64:128, :], iv[:, r0 + 1:r0 + R_C + 3, :])
    it = in_t.rearrange("p (r w) -> p r w", w=WP)
    dv = dst.rearrange("c (r w) -> c r w", w=WP)

    # groups of tile-pairs; each pair shares one [128,480] psum tile
    # (lo half = tile A, hi half = tile B -> concurrent col groups)
    groups = [[(0, 3, 3, 3), (6, 3, 9, 3)],
              [(12, 3, 15, 3), (18, 3, 21, 3)],
              [(24, 3, 27, 3), (30, 2, None, 0)]]
    for gi, grp in enumerate(groups):
        pairs = []
        for tlA, nrA, tlB, nrB in grp:
            ps = p_ps.tile([128, 480], F32, name=f"ps{sid}_{band}_{tlA}",
                           tag="ps")
            pairs.append((tlA, nrA, tlB, nrB, ps))
        # kx-outer over the group's pairs; A->cols 0-63, B->cols 64-127
        for kx in range(3):
            for tlA, nrA, tlB, nrB, ps in pairs:
                nc.tensor.matmul(
                    ps[0:64, 0:nrA * W], wA[:, kx * M:(kx + 1) * M],
                    it[:, tlA:tlA + nrA, kx:kx + 160],
                    start=(kx == 0), stop=False)
                if tlB is not None:
                    nc.tensor.matmul(
                        ps[64:128, 0:nrB * W], wA[:, kx * M:(kx + 1) * M],
                        it[:, tlB:tlB + nrB, kx:kx + 160],
                        start=(kx == 0), stop=False)
        for kx in range(3):
            for tlA, nrA, tlB, nrB, ps in pairs:
                nc.tensor.matmul(
                    ps[0:64, 0:nrA * W], wB[0:64, kx * M:(kx + 1) * M],
                    it[0:64, tlA + 2:tlA + 2 + nrA, kx:kx + 160],
                    start=False, stop=(kx == 2))
                if tlB is not None:
                    nc.tensor.matmul(
                        ps[64:128, 0:nrB * W], wB[0:64, kx * M:(kx + 1) * M],
                        it[0:64, tlB + 2:tlB + 2 + nrB, kx:kx + 160],
                        start=False, stop=(kx == 2))

        for tlA, nrA, tlB, nrB, ps in pairs:
            ev = p_ev.tile([128, 3 * WP], F16, name=f"ev{sid}_{band}_{tlA}",
                           tag="evP")
            e3 = ev.rearrange("p (r w) -> p r w", w=WP)
            nrm = max(nrA, nrB)
            nc.gpsimd.memset(e3[:, 0:nrm, 0:1], 0.0)
            nc.gpsimd.memset(e3[:, 0:nrm, 161:162], 0.0)
            halves = [(0, tlA, nrA)]
            if tlB is not None:
                halves.append((64, tlB, nrB))
            for p0, tl, nr in halves:
                N = nr * W
                dst_i = e3[p0:p0 + 64, 0:nr, 1:161]
                if kind == "lrelu":
                    # Prelu honors alpha; Lrelu's slope is hardwired 0.01
                    nc.scalar.activation(dst_i, ps[p0:p0 + 64, 0:N],
                                         ACTF.Prelu, alpha=0.1)
                else:  # "padded"
                    nc.scalar.activation(dst_i, ps[p0:p0 + 64, 0:N],
                                         ACTF.Copy)
                pr = r0 + tl + 1
                nc.sync.dma_start(dv[:, pr:pr + nr, :],
                                  ev[p0:p0 + 64, 0:nr * WP])


def _conv_band_m128(nc, pools, src, dst, wA, wB, kind, band, sid):
    """One 32-row band of a 3x3 conv stage with Cout=128 (M=128)."""
    p_in, p_ps, p_ev = pools
    M = 128
    r0 = band * R_C
    in_t = p_in.tile([128, (R_C + 2) * WP], F16, name=f"cin{sid}_{band}",
                     tag="cin")
    iv = src.rearrange("c (r w) -> c r w", w=WP)
    nc.scalar.dma_start(in_t[0:64, :], iv[:, r0:r0 + R_C + 2, :])
    nc.scalar.dma_start(in_t[64:128, :], iv[:, r0 + 1:r0 + R_C + 3, :])
    it = in_t.rearrange("p (r w) -> p r w", w=WP)

    groups = [[(0, 3), (3, 3), (6, 3), (9, 3)],
              [(12, 3), (15, 3), (18, 3), (21, 3)],
              [(24, 3), (27, 3), (30, 2)]]
    for grp in groups:
        tiles = []
        for tl, nr in grp:
            ps = p_ps.tile([M, 480], F32, name=f"ps{sid}_{band}_{tl}",
                           tag="ps")
            tiles.append((tl, nr, ps))
        for kx in range(3):
            for tl, nr, ps in tiles:
                nc.tensor.matmul(
                    ps[:, 0:nr * W], wA[:, kx * M:(kx + 1) * M],
                    it[:, tl:tl + nr, kx:kx + 160],
                    start=(kx == 0), stop=False)
        for kx in range(3):
            for tl, nr, ps in tiles:
                nc.tensor.matmul(
                    ps[:, 0:nr * W], wB[0:64, kx * M:(kx + 1) * M],
                    it[0:64, tl + 2:tl + 2 + nr, kx:kx + 160],
                    start=False, stop=(kx == 2))

        for tl, nr, ps in tiles:
            N = nr * W
            if kind == "offsets":
                offy_d, offx_d = dst
                ev = p_ev.tile([128, 480], F16, name=f"ev{sid}_{band}_{tl}",
                               tag="evO")
                nc.scalar.activation(ev[:, 0:N], ps[:, 0:N], ACTF.Copy)
                nc.sync.dma_start(
                    offy_d[:, (r0 + tl) * W:(r0 + tl + nr) * W], ev[0:64, 0:N])
                nc.sync.dma_start(
                    offx_d[:, (r0 + tl) * W:(r0 + tl + nr) * W],
                    ev[64:128, 0:N])
            else:  # "flat" f16 output
                ev = p_ev.tile([128, 480], F16, name=f"ev{sid}_{band}_{tl}",
                               tag="evF")
                nc.scalar.activation(ev[:, 0:N], ps[:, 0:N], ACTF.Copy)
                nc.sync.dma_start(
                    dst[:, (r0 + tl) * W:(r0 + tl + nr) * W], ev[:, 0:N])


def _bilinear_block(nc, bpools, hr16, offy_d, offx_d, samp_d, biases, blk):
    """One 32-row block of bilinear sampling (16 rows per partition half).

    Residual coords = raw offsets (no clipping): hr16 is replicate-padded,
    which makes the unclipped 5x5 hat window exactly equal to the
    reference's clipped bilinear for |off| <= 2.
    """
    p_off, p_s, p_w, p_hr, p_acc = bpools
    rA = 2 * R_B * blk
    oy = p_off.tile([128, FD], F16, name=f"oy{blk}", tag="oy")
    ox = p_off.tile([128, FD], F16, name=f"ox{blk}", tag="ox")
    for half, r in ((0, rA), (1, rA + R_B)):
        nc.gpsimd.dma_start(oy[64 * half:64 * half + 64, :],
                            offy_d[:, r * W:(r + R_B) * W])
        nc.gpsimd.dma_start(ox[64 * half:64 * half + 64, :],
                            offx_d[:, r * W:(r + R_B) * W])
    hr_t = p_hr.tile([128, (R_B + 4) * W16], F16, name=f"hr{blk}", tag="hr")
    hv = hr16.rearrange("c (r w) -> c r w", w=W16)
    for half, r in ((0, rA), (1, rA + R_B)):
        nc.gpsimd.dma_start(hr_t[64 * half:64 * half + 64, :],
                            hv[:, r:r + R_B + 4, :])
    ht = hr_t.rearrange("p (r w) -> p r w", w=W16)

    # negated hat weights: w'_d = min(|s-d|-1, 0) = -relu(1-|s-d|)
    DVALS = (-2, -1, 0, 1, 2)

    def build_w(src, i, ax):
        wt = p_w.tile([128, FD], F16, name=f"w{ax}{i}_{blk}", tag=f"w{ax}{i}")
        if i == 0:
            # d=-2: |s+2| = s+2 for s >= -2, so w' = min(s+1, 0); no Abs
            nc.vector.tensor_scalar(wt[:, :], src[:, :], 1.0, 0.0,
                                    ALU.add, ALU.min)
        else:
            a = p_s.tile([128, FD], F16, name=f"a{ax}{i}_{blk}", tag="abs")
            nc.scalar.activation(a[:, :], src[:, :], ACTF.Abs,
                                 bias=biases[DVALS[i]][:, :])
            nc.vector.tensor_scalar(wt[:, :], a[:, :], 1.0, 0.0,
                                    ALU.subtract, ALU.min)
        return wt

    wx = [build_w(ox, i, "x") for i in range(5)]
    wy = [build_w(oy, i, "y") for i in range(5)]

    cs = [p_acc.tile([128, FD], F16, name=f"c{k}_{blk}", tag=f"c{k}")
          for k in range(3)]
    tA = p_acc.tile([128, FD], F16, name=f"tA{blk}", tag="tA")
    tB = p_acc.tile([128, FD], F16, name=f"tB{blk}", tag="tB")
    tmp2 = p_acc.tile([128, FD], F16, name=f"tmp2{blk}", tag="tmp2")
    yp = [p_acc.tile([128, FD], F16, name=f"yp{k}_{blk}", tag=f"yp{k}")
          for k in range(4)]

    def hview(dy, dx):
        return ht[:, dy:dy + R_B, dx:dx + 160]

    # chain results: dy0->c0, dy1->c1, dy2->c2, dy3->c0, dy4->c1.
    # Pairs are interleaved op-by-op so every tensor_add's operands were
    # written >=2 instructions earlier (hides the DVE read-write bubble).
    def xchain2(dyA, cA, dyB, cB):
        for dx in range(5):
            if dx == 0:
                nc.vector.tensor_mul(cA[:, :], wx[0][:, :], hview(dyA, 0))
                nc.vector.tensor_mul(cB[:, :], wx[0][:, :], hview(dyB, 0))
            else:
                nc.vector.tensor_mul(tA[:, :], wx[dx][:, :], hview(dyA, dx))
                nc.vector.tensor_mul(tB[:, :], wx[dx][:, :], hview(dyB, dx))
                nc.vector.tensor_add(cA[:, :], cA[:, :], tA[:, :])
                nc.vector.tensor_add(cB[:, :], cB[:, :], tB[:, :])

    xchain2(0, cs[0], 1, cs[1])
    nc.gpsimd.tensor_mul(yp[0][:, :], wy[0][:, :], cs[0][:, :])
    nc.gpsimd.tensor_mul(yp[1][:, :], wy[1][:, :], cs[1][:, :])
    xchain2(2, cs[2], 3, cs[0])
    nc.gpsimd.tensor_mul(yp[2][:, :], wy[2][:, :], cs[2][:, :])
    nc.gpsimd.tensor_mul(yp[3][:, :], wy[3][:, :], cs[0][:, :])

    acc = p_acc.tile([128, R_B * WP], F16, name=f"acc{blk}", tag="acc")
    a3 = acc.rearrange("p (r w) -> p r w", w=WP)
    nc.gpsimd.memset(a3[:, :, 0:1], 0.0)
    nc.gpsimd.memset(a3[:, :, 161:162], 0.0)
    acc_i = a3[:, :, 1:161]

    # dy=4 chain interleaved with the yp reduction tree; final y-mul on DVE
    # so the tail never waits on Pool.
    c4 = cs[1]
    nc.vector.tensor_mul(c4[:, :], wx[0][:, :], hview(4, 0))
    nc.vector.tensor_add(tB[:, :], yp[0][:, :], yp[1][:, :])
    nc.vector.tensor_mul(tA[:, :], wx[1][:, :], hview(4, 1))
    nc.vector.tensor_add(c4[:, :], c4[:, :], tA[:, :])
    nc.vector.tensor_add(cs[2][:, :], yp[2][:, :], yp[3][:, :])
    nc.vector.tensor_mul(tA[:, :], wx[2][:, :], hview(4, 2))
    nc.vector.tensor_add(c4[:, :], c4[:, :], tA[:, :])
    nc.vector.tensor_add(tB[:, :], tB[:, :], cs[2][:, :])
    nc.vector.tensor_mul(tA[:, :], wx[3][:, :], hview(4, 3))
    nc.vector.tensor_add(c4[:, :], c4[:, :], tA[:, :])
    nc.vector.tensor_mul(tA[:, :], wx[4][:, :], hview(4, 4))
    nc.vector.tensor_add(c4[:, :], c4[:, :], tA[:, :])
    nc.vector.tensor_mul(tmp2[:, :], wy[4][:, :], c4[:, :])
    nc.vector.tensor_add(acc_i, tB[:, :], tmp2[:, :])

    nc.sync.dma_start(samp_d[:, (rA + 1) * WP:(rA + 1 + R_B) * WP],
                      acc[0:64, :])
    nc.sync.dma_start(
        samp_d[:, (rA + R_B + 1) * WP:(rA + 2 * R_B + 1) * WP],
        acc[64:128, :])


def build_program(debug_outputs=False):
    ikind = "ExternalOutput" if debug_outputs else "Internal"
    nc = bacc.Bacc("TRN2", target_bir_lowering=False, debug=False,
                   num_devices=N_CORES)
    xpad = nc.dram_tensor("xpad", [C, HP * WP], F16, kind="ExternalInput").ap()
    hr16 = nc.dram_tensor("hr16", [C, H16 * W16], F16,
                          kind="ExternalInput").ap()
    w_in = {}
    for s, m in (("w1", 64), ("w2", 64), ("w3", 64), ("wo", 128)):
        w_in[s + "A"] = nc.dram_tensor(s + "A", [128, 3 * m], F16,
                                       kind="ExternalInput").ap()
        w_in[s + "B"] = nc.dram_tensor(s + "B", [64, 3 * m], F16,
                                       kind="ExternalInput").ap()
    w_in["wcA"] = nc.dram_tensor("wcA", [128, 3 * 128], F16,
                                 kind="ExternalInput").ap()
    w_in["wcB"] = nc.dram_tensor("wcB", [64, 3 * 128], F16,
                                 kind="ExternalInput").ap()

    out = nc.dram_tensor("out", [128, HW], F16, kind="ExternalOutput").ap()

    h1p = nc.dram_tensor("h1p", [C, HP * WP], F16, kind=ikind).ap()
    h2p = nc.dram_tensor("h2p", [C, HP * WP], F16, kind=ikind).ap()
    estp = nc.dram_tensor("estp", [C, HP * WP], F16, kind=ikind).ap()
    offy_d = nc.dram_tensor("offy", [C, HW], F16, kind=ikind).ap()
    offx_d = nc.dram_tensor("offx", [C, HW], F16, kind=ikind).ap()
    samp_d = nc.dram_tensor("samp", [C, HP * WP], F16, kind=ikind).ap()

    with ExitStack() as ctx:
        tc = ctx.enter_context(tile.TileContext(nc))
        p_const = ctx.enter_context(tc.tile_pool(name="const", bufs=1))

        zrow = p_const.tile([64, 3 * WP], F32, name="zrow")
        nc.vector.memset(zrow[:, :], 0.0)
        zrow16 = zrow.bitcast(F16)
        for buf in (h1p, h2p, estp, samp_d):
            bv = buf.rearrange("c (r w) -> c r w", w=WP)
            nc.sync.dma_start(bv[:, 0:1, :], zrow16[:, 0:WP])
            nc.sync.dma_start(bv[:, 161:164, :], zrow16[:, 0:3 * WP])

        wsb = {}
        for name, ap in w_in.items():
            t = p_const.tile(list(ap.shape), ap.dtype, name="w_" + name)
            nc.sync.dma_start(t[:, :], ap[:, :])
            wsb[name] = t
        biases = {}
        for d in (-2, -1, 0, 1, 2):
            bt = p_const.tile([128, 1], F32, name=f"bias{d}")
            nc.vector.memset(bt[:, :], float(-d))
            biases[d] = bt

        p_in = ctx.enter_context(tc.tile_pool(name="c_in", bufs=3))
        p_ps = ctx.enter_context(tc.tile_pool(name="c_ps", bufs=8,
                                              space="PSUM"))
        p_ev = ctx.enter_context(tc.tile_pool(name="c_ev", bufs=4))
        pools = (p_in, p_ps, p_ev)

        p_off = ctx.enter_context(tc.tile_pool(name="b_off", bufs=2))
        p_s = ctx.enter_context(tc.tile_pool(name="b_s", bufs=2))
        p_w = ctx.enter_context(tc.tile_pool(name="b_w", bufs=1))
        p_hr = ctx.enter_context(tc.tile_pool(name="b_hr", bufs=2))
        p_acc = ctx.enter_context(tc.tile_pool(name="b_acc", bufs=1))
        bpools = (p_off, p_s, p_w, p_hr, p_acc)

        # stage s processes band i - s at iteration i; issue order puts the
        # final conv first (its deps resolved last iteration) and the big
        # DVE bilinear chain last.
        stages = {
            0: lambda b: _conv_band_m64(nc, pools, xpad, h1p, wsb["w1A"],
                                        wsb["w1B"], "lrelu", b, 1),
            1: lambda b: _conv_band_m64(nc, pools, h1p, h2p, wsb["w2A"],
                                        wsb["w2B"], "lrelu", b, 2),
            2: lambda b: _conv_band_m64(nc, pools, h2p, estp, wsb["w3A"],
                                        wsb["w3B"], "padded", b, 3),
            3: lambda b: _conv_band_m128(nc, pools, estp, (offy_d, offx_d),
                                         wsb["woA"], wsb["woB"], "offsets",
                                         b, 4),
            4: lambda b: _bilinear_block(nc, bpools, hr16, offy_d, offx_d,
                                         samp_d, biases, b),
            6: lambda b: _conv_band_m128(nc, pools, samp_d, out, wsb["wcA"],
                                         wsb["wcB"], "flat", b, 6),
        }
        issue_order = [6, 0, 1, 2, 3, 4]
        n_stage = 7
        for i in range(N_BAND + n_stage - 1):
            for s in issue_order:
                b = i - s
                if 0 <= b < N_BAND:
                    stages[s](b)
    nc.compile()
    return nc


def _prep_weights(w, m, dtype):
    # w: (Cout, Cin, 3, 3) -> wA [128, 3*m] (ky=0/1 K-paired), wB [64, 3*m]
    wA = np.zeros((128, 3 * m), dtype=dtype)
    wB = np.zeros((64, 3 * m), dtype=dtype)
    for kx in range(3):
        wA[0:64, kx * m:(kx + 1) * m] = w[:, :, 0, kx].T
        wA[64:128, kx * m:(kx + 1) * m] = w[:, :, 1, kx].T
        wB[:, kx * m:(kx + 1) * m] = w[:, :, 2, kx].T
    return wA, wB


def _host_inputs(inputs):
    hr = np.asarray(inputs["hr_features"], dtype=np.float32)
    shared = {}
    for s, key, m in (("w1", "est_w1", 64), ("w2", "est_w2", 64),
                      ("w3", "est_w3", 64)):
        A, Bm = _prep_weights(np.asarray(inputs[key], np.float32), m,
                              np.float16)
        shared[s + "A"], shared[s + "B"] = A, Bm
    # offset conv: permute output channels to [dy c=0..63 | dx c=0..63]
    wo = np.asarray(inputs["offset_w"], np.float32)
    perm = np.concatenate([np.arange(0, 128, 2), np.arange(1, 128, 2)])
    A, Bm = _prep_weights(wo[perm], 128, np.float16)
    shared["woA"], shared["woB"] = A, Bm
    A, Bm = _prep_weights(np.asarray(inputs["conv1_w"], np.float32), 128,
                          np.float16)
    shared["wcA"], shared["wcB"] = A, Bm

    in_maps = []
    for b in range(B):
        m = dict(shared)
        xpad = np.zeros((C, HP, WP), np.float16)
        xpad[:, 1:161, 1:161] = hr[b]
        m["xpad"] = xpad.reshape(C, HP * WP)
        # replicate padding: unclipped hat-window == clipped bilinear
        hr16 = np.pad(hr[b].astype(np.float16), ((0, 0), (2, 2), (2, 2)),
                      mode="edge")
        m["hr16"] = hr16.reshape(C, H16 * W16)
        in_maps.append(m)
    return in_maps


def kernel(**inputs):
    if "nc" not in _CACHE:
        _CACHE["nc"] = build_program()
    nc = _CACHE["nc"]
    in_maps = _host_inputs(inputs)
    res = run_bass_kernel_spmd(nc, in_maps, list(range(N_CORES)))
    out = np.stack([res.results[b]["out"].reshape(128, H, W)
                    for b in range(B)])
    return out.astype(np.float32)
